# revision 30
# baseline (speedup 1.0000x reference)
"""AttentionBlock3D (B=4, C=256, D=H=W=16) on 8 NeuronCores.

Sharding: core c handles batch b = c//2, query-half h = c%2. Each core's
input is x[b] with the spatial axis rotated so its 2048 query positions sit
at columns 0..2047 (softmax/attention are permutation-invariant over keys,
so k/v/groupnorm stats computed from the rotated tensor are unchanged).

Per-core kernel (SPMD, identical program):
  - GroupNorm folded into qkv weights: h = a*x + b per channel, so
    qkv = (W*a).T.T @ x with an adjusted bias (computed on-chip via tiny
    matmuls, since a/b depend on the per-batch group statistics).
  - q, k in (C, N) layout; v^T in (N, C) layout (x as stationary operand).
  - scores computed transposed: s_T[nk, nq] = k.T q, exp on ScalarE with
    the 1/sqrt(C) scale folded in, no max-subtraction (scores are O(1)).
  - o_unnorm = v^T.T @ exp_s accumulated over 32 key tiles in PSUM;
    softmax denominators accumulated on DVE/GPSIMD (alternating), then an
    all-ones matmul gives column sums broadcast to 128 partitions, and the
    normalization is applied AFTER proj (column scaling commutes with the
    channel matmul).
  - v-bias folded into the proj bias (softmax rows sum to 1); group-stat
    bias corrections folded into host-precomputed G/PG matrices so only
    K=8 matmuls against the 8 group scalars run on device.
Scores/AV run bf16 (errors average out across 4096 attention terms), proj
runs float32r, the residual path stays full fp32.
"""

import os
import sys

if "/opt/trn_rl_repo" not in sys.path:
    sys.path.insert(0, "/opt/trn_rl_repo")

import ml_dtypes
import numpy as np

# run_bass_kernel_spmd honors BASS_TRACE, but NTFF tracing needs the
# antenv.axon_hooks registry, which this image lacks unless it has been
# injected (see ntff_hook.py). Register it if possible; otherwise make sure
# a stray BASS_TRACE can't break the run.
try:
    import ntff_hook  # noqa: F401
except Exception:
    os.environ["BASS_NEVER_TRACE"] = "1"

import concourse.bass as bass
import concourse.mybir as mybir
import concourse.tile as tile
from concourse import bacc
from concourse.bass import ds, ts
from concourse.bass_utils import run_bass_kernel_spmd

B, C, D, H, W = 4, 256, 16, 16, 16
N = D * H * W  # 4096
NQ = N // 2  # 2048 queries per core
G = 8  # groups
NG_ELEMS = (C // G) * N  # elements per (batch, group)
EPS = 1e-5
SCALE = C ** (-0.5)
N_CORES = 8

F32 = mybir.dt.float32
FR = mybir.dt.float32r
BF = mybir.dt.bfloat16
AF = mybir.ActivationFunctionType
AX = mybir.AxisListType

LAST_RESULT = None  # BassKernelResults of the most recent run (for test harness)
_CACHED_NC = None


def _fr(ap):
    return ap.bitcast(FR)


def _emit(tc, aps):
    from contextlib import ExitStack

    nc = tc.nc
    x_d, xr_d, wt_d, wpt_d, g_d, pg_d, cstq_d, cstp_d, gam_d, mf_d, mt_d, out_d = aps

    with ExitStack() as ctx:
        const = ctx.enter_context(tc.tile_pool(name="const", bufs=1))
        big = ctx.enter_context(tc.tile_pool(name="big", bufs=1))
        expp = ctx.enter_context(tc.tile_pool(name="expp", bufs=6))
        osb = ctx.enter_context(tc.tile_pool(name="osb", bufs=6))
        outp = ctx.enter_context(tc.tile_pool(name="outp", bufs=4))
        small = ctx.enter_context(tc.tile_pool(name="small", bufs=2))
        accp = ctx.enter_context(tc.tile_pool(name="accp", bufs=6))
        scr = ctx.enter_context(tc.tile_pool(name="scr", bufs=2))
        ps_s = ctx.enter_context(tc.tile_pool(name="ps_s", bufs=3, space="PSUM"))
        ps_o = ctx.enter_context(tc.tile_pool(name="ps_o", bufs=2, space="PSUM"))
        ps_bc = ctx.enter_context(tc.tile_pool(name="ps_bc", bufs=1, space="PSUM"))
        ps_p = ctx.enter_context(tc.tile_pool(name="ps_p", bufs=2, space="PSUM"))

        # ---- x DMA first (stats gate everything) in 512-col chunks, groupnorm stats accumulated per chunk ----
        xs = []
        sqp, sqq = [], []
        for ci in range(2):
            xs.append(big.tile([128, N], BF, tag=f"x{ci}", name=f"x{ci}"))
            sqp.append(const.tile([128, 4], F32, tag=f"sqp{ci}", name=f"sqp{ci}"))
            sqq.append(const.tile([128, 4], F32, tag=f"sqq{ci}", name=f"sqq{ci}"))
        for c in range(4):
            for ci in range(2):
                nc.sync.dma_start(xs[ci][:, ts(c, 1024)], x_d[ts(ci, 128), ts(c, 1024)])
                chunk = xs[ci][:, ts(c, 1024)]
                nc.vector.reduce_sum(sqp[ci][:, c : c + 1], chunk, axis=AX.X)
                sc_t = scr.tile([128, 1024], F32, tag="sc", name="sc")
                nc.scalar.activation(
                    sc_t[:], chunk, AF.Square, accum_out=sqq[ci][:, c : c + 1]
                )

        # ---- weights / consts ----
        wt_raw = []
        for ci in range(2):
            t = const.tile([128, 3 * C], BF, tag=f"wtr{ci}", name=f"wtr{ci}")
            nc.sync.dma_start(t[:], wt_d[ts(ci, 128), :])
            wt_raw.append(t)
        wpt_fr = []
        for ci in range(2):
            t = const.tile([128, C], F32, tag=f"wpt{ci}", name=f"wpt{ci}")
            nc.sync.dma_start(t[:], wpt_d[ts(ci, 128), :])
            tf = const.tile([128, C], FR, tag=f"wptf{ci}", name=f"wptf{ci}")
            nc.scalar.activation(tf[:], t[:], AF.Copy)
            wpt_fr.append(tf)
        g_sb = const.tile([8, 3 * C], F32, tag="g_sb", name="g_sb")
        nc.sync.dma_start(g_sb[:], g_d[:])
        pg_sb = const.tile([8, C], F32, tag="pg_sb", name="pg_sb")
        nc.sync.dma_start(pg_sb[:], pg_d[:])
        cstq_sb = const.tile([128, 6], F32, tag="cstq", name="cstq")
        nc.sync.dma_start(cstq_sb[:], cstq_d[:])
        cstp_sb = const.tile([128, 2], F32, tag="cstp", name="cstp")
        nc.sync.dma_start(cstp_sb[:], cstp_d[:])
        gam_sb = const.tile([128, 2], F32, tag="gam", name="gam")
        nc.sync.dma_start(gam_sb[:], gam_d[:])
        mf_sb = const.tile([128, 16], F32, tag="mf", name="mf")
        nc.sync.dma_start(mf_sb[:], mf_d[:])
        mt_sb = const.tile([8, 256], F32, tag="mt", name="mt")
        nc.sync.dma_start(mt_sb[:], mt_d[:])

        ones = const.tile([128, 128], F32, tag="ones", name="ones")
        nc.vector.memset(ones[:], 1.0)
        ones_bf = const.tile([128, 128], BF, tag="ones_bf", name="ones_bf")
        nc.vector.memset(ones_bf[:], 1.0)

        warm_ps = ps_bc.tile([128, 512], F32, tag="bc", name="warm")
        n_warm = 216
        for i in range(n_warm):
            nc.tensor.matmul(
                warm_ps[:, 0:128], ones_bf[:], ones_bf[:],
                start=(i == 0), stop=(i == n_warm - 1),
            )
        warm_sink = const.tile([1, 1], F32, tag="warm_sink", name="warm_sink")
        nc.vector.tensor_copy(warm_sink[:], warm_ps[0:1, 0:1])

        sq = []
        for ci in range(2):
            t = const.tile([128, 2], F32, tag=f"sq{ci}", name=f"sq{ci}")  # [sum, sumsq]
            nc.vector.reduce_sum(t[:, 0:1], sqp[ci][:], axis=AX.X)
            nc.vector.reduce_sum(t[:, 1:2], sqq[ci][:], axis=AX.X)
            sq.append(t)

        gs_ps = ps_p.tile([8, 2], F32, tag="p", name="p")  # group [sum, sumsq]
        for ci in range(2):
            nc.tensor.matmul(
                gs_ps[:], mf_sb[:, ds(8 * ci, 8)], sq[ci][:],
                start=(ci == 0), stop=(ci == 1),
            )
        dum = const.tile([8, 1], F32, tag="dum", name="dum")
        nc.scalar.activation(dum[:], gs_ps[:, 0:1], AF.Sqrt, scale=0.0, bias=1.0)
        stats = const.tile([8, 2], F32, tag="stats", name="stats")  # [mean, rstd]
        tmp8 = const.tile([8, 2], F32, tag="tmp8", name="tmp8")  # [mean^2, E[x^2]]
        inv_ng = 1.0 / NG_ELEMS
        nc.vector.tensor_scalar_mul(stats[:, 0:1], gs_ps[:, 0:1], inv_ng)
        nc.vector.tensor_scalar_mul(tmp8[:, 1:2], gs_ps[:, 1:2], inv_ng)
        nc.vector.tensor_mul(tmp8[:, 0:1], stats[:, 0:1], stats[:, 0:1])
        var8 = const.tile([8, 1], F32, tag="var8", name="var8")
        nc.vector.tensor_sub(var8[:], tmp8[:, 1:2], tmp8[:, 0:1])
        nc.vector.tensor_scalar_add(var8[:], var8[:], EPS)
        sd8 = const.tile([8, 1], F32, tag="sd8", name="sd8")
        nc.scalar.activation(sd8[:], var8[:], AF.Sqrt)
        nc.vector.reciprocal(stats[:, 1:2], sd8[:])

        # broadcast rstd to channels; per-channel scale a = gamma * rstd
        m8 = const.tile([8, 1], F32, tag="m8", name="m8")
        nc.vector.tensor_mul(m8[:], stats[:, 0:1], stats[:, 1:2])
        a_sb = []
        for ci in range(2):
            ch_ps = ps_p.tile([128, 1], F32, tag="p", name="p")
            nc.tensor.matmul(
                ch_ps[:], mt_sb[:, ts(ci, 128)], stats[:, 1:2], start=True, stop=True
            )
            a_t = const.tile([128, 1], F32, tag=f"a{ci}", name=f"a{ci}")
            nc.vector.tensor_mul(a_t[:], gam_sb[:, ci : ci + 1], ch_ps[:])
            a_sb.append(a_t)

        # scale qkv weights by a (per input channel = partition)
        wts = []
        for ci in range(2):
            t = const.tile([128, 3 * C], BF, tag=f"wts{ci}", name=f"wts{ci}")
            if ci == 0:
                nc.scalar.activation(t[:], wt_raw[ci][:], AF.Copy, scale=a_sb[ci][:])
            else:
                nc.vector.tensor_scalar_mul(t[:], wt_raw[ci][:], a_sb[ci][:])
            wts.append(t)

        # ---- qkv projections (bias matmuls interleaved after the first 4
        # tiles so the PE isn't serialized on the tiny bias chain) ----
        q_sb, k_sb = [], []
        for ci in range(2):
            q_sb.append(big.tile([128, NQ], BF, tag=f"q{ci}", name=f"q{ci}"))
            k_sb.append(big.tile([128, N], BF, tag=f"k{ci}", name=f"k{ci}"))
        plans = [
            (0, q_sb[0], NQ), (1, q_sb[1], NQ),
            (2, k_sb[0], N), (3, k_sb[1], N),
        ]
        jobs = [(j, dst, nt) for j, dst, ncols in plans for nt in range(ncols // 512)]

        def qkv_mm(idx):
            j, dst, nt = jobs[idx]
            pool = ps_s if idx % 2 == 0 else ps_o
            ptag = "s" if idx % 2 == 0 else "o"
            ps = pool.tile([128, 512], F32, tag=ptag, name=ptag)
            for ci in range(2):
                nc.tensor.matmul(
                    ps[:], wts[ci][:, ts(j, 128)],
                    xs[ci][:, ts(nt, 512)],
                    start=(ci == 0), stop=(ci == 1),
                )
            return ps

        def qkv_evac(idx, ps):
            j, dst, nt = jobs[idx]
            if idx % 2 == 0:
                nc.scalar.activation(
                    dst[:, ts(nt, 512)], ps[:], AF.Identity,
                    bias=qb_eff[:, j : j + 1],
                )
            else:
                nc.vector.tensor_scalar_add(
                    dst[:, ts(nt, 512)], ps[:], qb_eff[:, j : j + 1]
                )

        qb_eff = const.tile([128, 6], F32, tag="qb_eff", name="qb_eff")
        head = [qkv_mm(i) for i in range(4)]

        # effective biases: cst - sum_g (mean_g*rstd_g) * G[g, :]
        bb_ps = ps_p.tile([128, 6], F32, tag="p", name="p")
        for j in range(6):
            nc.tensor.matmul(
                bb_ps[:, j : j + 1], g_sb[:, ts(j, 128)], m8[:],
                start=True, stop=True,
            )
        nc.vector.tensor_sub(qb_eff[:], cstq_sb[:], bb_ps[:])
        pbps = ps_p.tile([128, 2], F32, tag="p", name="p")
        for ob in range(2):
            nc.tensor.matmul(
                pbps[:, ob : ob + 1], pg_sb[:, ts(ob, 128)], m8[:],
                start=True, stop=True,
            )
        pb_eff = const.tile([128, 2], F32, tag="pb_eff", name="pb_eff")
        nc.vector.tensor_sub(pb_eff[:], cstp_sb[:], pbps[:])

        for i in range(4):
            qkv_evac(i, head[i])
        for idx in range(4, len(jobs)):
            ps = qkv_mm(idx)
            qkv_evac(idx, ps)

        # v^T: (nk, v-channel) layout, no bias (folded into proj bias)
        vt_sb = big.tile([128, 32, 256], BF, tag="vt", name="vt")
        for t in range(32):
            pool = ps_s if t % 2 == 0 else ps_o
            ptag = "s" if t % 2 == 0 else "o"
            ps = pool.tile([128, 512], F32, tag=ptag, name=ptag)
            for ci in range(2):
                nc.tensor.matmul(
                    ps[:, 0:256], xs[ci][:, ts(t, 128)],
                    wts[ci][:, ds(512, 256)],
                    start=(ci == 0), stop=(ci == 1),
                )
            if t % 2 == 0:
                nc.vector.tensor_copy(vt_sb[:, t, :], ps[:, 0:256])
            else:
                nc.scalar.activation(vt_sb[:, t, :], ps[:, 0:256], AF.Copy)

        # x + pb_eff precomputed for the residual tail
        xpb = []
        for ob in range(2):
            xr_t = big.tile([128, NQ], F32, tag=f"xr{ob}", name=f"xr{ob}")
            nc.sync.dma_start(xr_t[:], xr_d[ts(ob, 128), :])
            t = big.tile([128, NQ], F32, tag=f"xpb{ob}", name=f"xpb{ob}")
            nc.scalar.activation(
                t[:], xr_t[:], AF.Identity, bias=pb_eff[:, ob : ob + 1]
            )
            xpb.append(t)

        # ---- attention + proj, per block of 512 queries ----
        for nqb in range(4):
            o_ps = [ps_o.tile([128, 512], F32, tag="o", name="o") for _ in range(2)]
            acc = [accp.tile([128, 512], BF, tag="acc", name="acc") for _ in range(2)]
            for t in range(32):
                s_ps = ps_s.tile([128, 512], F32, tag="s", name="s")
                for ci in range(2):
                    nc.tensor.matmul(
                        s_ps[:], k_sb[ci][:, ts(t, 128)],
                        q_sb[ci][:, ts(nqb, 512)],
                        start=(ci == 0), stop=(ci == 1),
                    )
                e_t = expp.tile([128, 512], BF, tag="e", name="e")
                nc.scalar.activation(e_t[:], s_ps[:], AF.Exp, scale=SCALE)
                first, last = (t == 0), (t == 31)
                for c2 in range(2):
                    nc.tensor.matmul(
                        o_ps[c2][:], vt_sb[:, t, ds(128 * c2, 128)],
                        e_t[:], start=first, stop=last,
                    )
                # denominator partials on DVE (two chains to halve latency)
                ef = e_t[:]
                eng = nc.vector if t % 2 == 0 else nc.gpsimd
                if t < 2:
                    eng.tensor_copy(acc[t][:], ef)
                else:
                    a = acc[t % 2]
                    eng.tensor_add(a[:], a[:], ef)
            nc.vector.tensor_add(acc[0][:], acc[0][:], acc[1][:])
            # unnormalized attention out -> SBUF, proj (normalization commutes)
            o_t = [osb.tile([128, 512], FR, tag="ob", name="ob") for _ in range(2)]
            nc.scalar.activation(o_t[0][:], o_ps[0][:], AF.Copy)
            nc.vector.tensor_copy(o_t[1][:], o_ps[1][:])
            p_ps = []
            for ob in range(2):
                pp = ps_p.tile([128, 512], F32, tag="p", name="p")
                for c2 in range(2):
                    nc.tensor.matmul(
                        pp[:], wpt_fr[c2][:, ts(ob, 128)], o_t[c2][:],
                        start=(c2 == 0), stop=(c2 == 1),
                    )
                p_ps.append(pp)
            # denominators: all-ones matmul = column sums broadcast to all
            # partitions in one shot; reciprocal on the full 128-lane tile
            bc_ps = ps_bc.tile([128, 512], F32, tag="bc", name="bc")
            nc.tensor.matmul(bc_ps[:], ones_bf[:], acc[0][:], start=True, stop=True)
            bc_sb = osb.tile([128, 512], F32, tag="bcs", name="bcs")
            nc.vector.reciprocal_approx_fast(bc_sb[:], bc_ps[:])
            for ob in range(2):
                r_t = outp.tile([128, 512], F32, tag="r", name="r")
                nc.vector.tensor_mul(r_t[:], p_ps[ob][:], bc_sb[:])
                f_t = outp.tile([128, 512], F32, tag="f", name="f")
                nc.vector.tensor_add(f_t[:], r_t[:], xpb[ob][:, ts(nqb, 512)])
                nc.sync.dma_start(out_d[ts(ob, 128), ts(nqb, 512)], f_t[:])


def _build():
    global _CACHED_NC
    if _CACHED_NC is not None:
        return _CACHED_NC
    nc = bacc.Bacc("TRN2", debug=False, target_bir_lowering=False)
    x_d = nc.dram_tensor("x", [C, N], BF, kind="ExternalInput").ap()
    xr_d = nc.dram_tensor("xr", [C, NQ], F32, kind="ExternalInput").ap()
    wt_d = nc.dram_tensor("wt", [C, 3 * C], BF, kind="ExternalInput").ap()
    wpt_d = nc.dram_tensor("wpt", [C, C], F32, kind="ExternalInput").ap()
    g_d = nc.dram_tensor("gmat", [8, 3 * C], F32, kind="ExternalInput").ap()
    pg_d = nc.dram_tensor("pgmat", [8, C], F32, kind="ExternalInput").ap()
    cstq_d = nc.dram_tensor("cstq", [128, 6], F32, kind="ExternalInput").ap()
    cstp_d = nc.dram_tensor("cstp", [128, 2], F32, kind="ExternalInput").ap()
    gam_d = nc.dram_tensor("gam", [128, 2], F32, kind="ExternalInput").ap()
    mf_d = nc.dram_tensor("maskf", [128, 16], F32, kind="ExternalInput").ap()
    mt_d = nc.dram_tensor("maskt", [8, 256], F32, kind="ExternalInput").ap()
    out_d = nc.dram_tensor("out", [C, NQ], F32, kind="ExternalOutput").ap()
    aps = (x_d, xr_d, wt_d, wpt_d, g_d, pg_d, cstq_d, cstp_d, gam_d, mf_d, mt_d, out_d)
    with tile.TileContext(nc) as tc:
        _emit(tc, aps)
    nc.compile()
    _CACHED_NC = nc
    return nc


def kernel(x, gn_gamma, gn_beta, qkv_w, qkv_b, proj_w, proj_b):
    global LAST_RESULT
    x = np.asarray(x, dtype=np.float32)
    gn_gamma = np.asarray(gn_gamma, dtype=np.float32)
    gn_beta = np.asarray(gn_beta, dtype=np.float32)
    qkv_w = np.asarray(qkv_w, dtype=np.float32)
    qkv_b = np.asarray(qkv_b, dtype=np.float32)
    proj_w = np.asarray(proj_w, dtype=np.float32)
    proj_b = np.asarray(proj_b, dtype=np.float32)

    xf = np.ascontiguousarray(x.reshape(B, C, N))
    wt = np.ascontiguousarray(qkv_w.T).astype(ml_dtypes.bfloat16)  # (C, 3C)
    wpt = np.ascontiguousarray(proj_w.T)  # (C, C) fp32 (fp32r on device)
    gam = np.ascontiguousarray(gn_gamma.reshape(2, 128).T)

    # host-folded bias constants:
    #   b_eff = cst - sum_g (mean_g * rstd_g) * G[g, :]
    # with G[g, o] = sum_{c in g} qkv_w[o, c] * gamma_c and
    # cst = qkv_b + qkv_w @ beta. Proj bias gets the same treatment through
    # proj_w (softmax rows sum to 1, so the v-bias passes through attention).
    grp_size = C // G
    gmat = np.zeros((G, 3 * C), np.float32)
    for g in range(G):
        sl = slice(g * grp_size, (g + 1) * grp_size)
        gmat[g] = qkv_w[:, sl] @ gn_gamma[sl]
    cst_qkv = qkv_b + qkv_w @ gn_beta  # (768,)
    pgmat = np.ascontiguousarray(gmat[:, 2 * C :] @ proj_w.T)  # (8, 256)
    cst_pb = proj_b + proj_w @ cst_qkv[2 * C :]  # (256,)
    cstq = np.ascontiguousarray(cst_qkv.reshape(6, 128).T)
    cstp = np.ascontiguousarray(cst_pb.reshape(2, 128).T)

    # group-membership masks (channels-per-partition <-> groups)
    ch = np.arange(C)
    grp = ch // (C // G)  # (256,)
    mf = np.zeros((128, 16), np.float32)  # [c_lo, ci*8 + g]
    for ci in range(2):
        for c_lo in range(128):
            mf[c_lo, ci * 8 + grp[ci * 128 + c_lo]] = 1.0
    mt = np.zeros((8, 256), np.float32)  # [g, c]
    mt[grp, ch] = 1.0

    in_maps = []
    for core in range(N_CORES):
        b, h = core // 2, core % 2
        xb = xf[b]
        if h:
            xc = np.ascontiguousarray(np.concatenate([xb[:, NQ:], xb[:, :NQ]], axis=1))
        else:
            xc = xb
        in_maps.append(
            {
                "x": xc.astype(ml_dtypes.bfloat16),
                "xr": np.ascontiguousarray(xc[:, :NQ]),
                "wt": wt, "wpt": wpt, "gmat": gmat, "pgmat": pgmat,
                "cstq": cstq, "cstp": cstp,
                "gam": gam, "maskf": mf, "maskt": mt,
            }
        )

    nc = _build()
    res = run_bass_kernel_spmd(nc, in_maps, core_ids=list(range(N_CORES)))
    LAST_RESULT = res

    out = np.empty((B, C, N), np.float32)
    for core in range(N_CORES):
        b, h = core // 2, core % 2
        out[b][:, h * NQ : (h + 1) * NQ] = res.results[core]["out"]
    return out.reshape(B, C, D, H, W)


# revision 31
# speedup vs baseline: 1.0151x; 1.0151x over previous
"""AttentionBlock3D (B=4, C=256, D=H=W=16) on 8 NeuronCores.

Sharding: core c handles batch b = c//2, query-half h = c%2. Each core's
input is x[b] with the spatial axis rotated so its 2048 query positions sit
at columns 0..2047 (softmax/attention are permutation-invariant over keys,
so k/v/groupnorm stats computed from the rotated tensor are unchanged).

Per-core kernel (SPMD, identical program):
  - GroupNorm folded into qkv weights: h = a*x + b per channel, so
    qkv = (W*a).T.T @ x with an adjusted bias (computed on-chip via tiny
    matmuls, since a/b depend on the per-batch group statistics).
  - q, k in (C, N) layout; v^T in (N, C) layout (x as stationary operand).
  - scores computed transposed: s_T[nk, nq] = k.T q, exp on ScalarE with
    the 1/sqrt(C) scale folded in, no max-subtraction (scores are O(1)).
  - o_unnorm = v^T.T @ exp_s accumulated over 32 key tiles in PSUM;
    softmax denominators accumulated on DVE/GPSIMD (alternating), then an
    all-ones matmul gives column sums broadcast to 128 partitions, and the
    normalization is applied AFTER proj (column scaling commutes with the
    channel matmul).
  - v-bias folded into the proj bias (softmax rows sum to 1); group-stat
    bias corrections folded into host-precomputed G/PG matrices so only
    K=8 matmuls against the 8 group scalars run on device.
Scores/AV run bf16 (errors average out across 4096 attention terms), proj
runs float32r, the residual path stays full fp32.
"""

import os
import sys

if "/opt/trn_rl_repo" not in sys.path:
    sys.path.insert(0, "/opt/trn_rl_repo")

import ml_dtypes
import numpy as np

# run_bass_kernel_spmd honors BASS_TRACE, but NTFF tracing needs the
# antenv.axon_hooks registry, which this image lacks unless it has been
# injected (see ntff_hook.py). Register it if possible; otherwise make sure
# a stray BASS_TRACE can't break the run.
try:
    import ntff_hook  # noqa: F401
except Exception:
    os.environ["BASS_NEVER_TRACE"] = "1"

import concourse.bass as bass
import concourse.mybir as mybir
import concourse.tile as tile
from concourse import bacc
from concourse.bass import ds, ts
from concourse.bass_utils import run_bass_kernel_spmd

B, C, D, H, W = 4, 256, 16, 16, 16
N = D * H * W  # 4096
NQ = N // 2  # 2048 queries per core
G = 8  # groups
NG_ELEMS = (C // G) * N  # elements per (batch, group)
EPS = 1e-5
SCALE = C ** (-0.5)
N_CORES = 8

F32 = mybir.dt.float32
FR = mybir.dt.float32r
BF = mybir.dt.bfloat16
AF = mybir.ActivationFunctionType
AX = mybir.AxisListType

LAST_RESULT = None  # BassKernelResults of the most recent run (for test harness)
_CACHED_NC = None


def _fr(ap):
    return ap.bitcast(FR)


def _emit(tc, aps):
    from contextlib import ExitStack

    nc = tc.nc
    x_d, xr_d, wt_d, wpt_d, g_d, pg_d, cstq_d, cstp_d, gam_d, mf_d, mt_d, out_d = aps

    with ExitStack() as ctx:
        const = ctx.enter_context(tc.tile_pool(name="const", bufs=1))
        big = ctx.enter_context(tc.tile_pool(name="big", bufs=1))
        expp = ctx.enter_context(tc.tile_pool(name="expp", bufs=6))
        osb = ctx.enter_context(tc.tile_pool(name="osb", bufs=6))
        outp = ctx.enter_context(tc.tile_pool(name="outp", bufs=4))
        small = ctx.enter_context(tc.tile_pool(name="small", bufs=2))
        accp = ctx.enter_context(tc.tile_pool(name="accp", bufs=6))
        scr = ctx.enter_context(tc.tile_pool(name="scr", bufs=2))
        ps_s = ctx.enter_context(tc.tile_pool(name="ps_s", bufs=3, space="PSUM"))
        ps_o = ctx.enter_context(tc.tile_pool(name="ps_o", bufs=2, space="PSUM"))
        ps_bc = ctx.enter_context(tc.tile_pool(name="ps_bc", bufs=1, space="PSUM"))
        ps_p = ctx.enter_context(tc.tile_pool(name="ps_p", bufs=2, space="PSUM"))

        # ---- x DMA first (stats gate everything) in 512-col chunks, groupnorm stats accumulated per chunk ----
        xs = []
        sqp, sqq = [], []
        for ci in range(2):
            xs.append(big.tile([128, N], BF, tag=f"x{ci}", name=f"x{ci}"))
            sqp.append(const.tile([128, 4], F32, tag=f"sqp{ci}", name=f"sqp{ci}"))
            sqq.append(const.tile([128, 4], F32, tag=f"sqq{ci}", name=f"sqq{ci}"))
        for c in range(4):
            for ci in range(2):
                nc.sync.dma_start(xs[ci][:, ts(c, 1024)], x_d[ts(ci, 128), ts(c, 1024)])
                chunk = xs[ci][:, ts(c, 1024)]
                nc.vector.reduce_sum(sqp[ci][:, c : c + 1], chunk, axis=AX.X)
                sc_t = scr.tile([128, 1024], F32, tag="sc", name="sc")
                nc.scalar.activation(
                    sc_t[:], chunk, AF.Square, accum_out=sqq[ci][:, c : c + 1]
                )

        # ---- weights / consts ----
        wt_raw = []
        for ci in range(2):
            t = const.tile([128, 3 * C], BF, tag=f"wtr{ci}", name=f"wtr{ci}")
            nc.sync.dma_start(t[:], wt_d[ts(ci, 128), :])
            wt_raw.append(t)
        wpt_fr = []
        for ci in range(2):
            t = const.tile([128, C], F32, tag=f"wpt{ci}", name=f"wpt{ci}")
            nc.sync.dma_start(t[:], wpt_d[ts(ci, 128), :])
            tf = const.tile([128, C], FR, tag=f"wptf{ci}", name=f"wptf{ci}")
            nc.scalar.activation(tf[:], t[:], AF.Copy)
            wpt_fr.append(tf)
        g_sb = const.tile([8, 3 * C], F32, tag="g_sb", name="g_sb")
        nc.sync.dma_start(g_sb[:], g_d[:])
        pg_sb = const.tile([8, C], F32, tag="pg_sb", name="pg_sb")
        nc.sync.dma_start(pg_sb[:], pg_d[:])
        cstq_sb = const.tile([128, 6], F32, tag="cstq", name="cstq")
        nc.sync.dma_start(cstq_sb[:], cstq_d[:])
        cstp_sb = const.tile([128, 2], F32, tag="cstp", name="cstp")
        nc.sync.dma_start(cstp_sb[:], cstp_d[:])
        gam_sb = const.tile([128, 2], F32, tag="gam", name="gam")
        nc.sync.dma_start(gam_sb[:], gam_d[:])
        mf_sb = const.tile([128, 16], F32, tag="mf", name="mf")
        nc.sync.dma_start(mf_sb[:], mf_d[:])
        mt_sb = const.tile([8, 256], F32, tag="mt", name="mt")
        nc.sync.dma_start(mt_sb[:], mt_d[:])

        ones = const.tile([128, 128], F32, tag="ones", name="ones")
        nc.vector.memset(ones[:], 1.0)
        ones_bf = const.tile([128, 128], BF, tag="ones_bf", name="ones_bf")
        nc.vector.memset(ones_bf[:], 1.0)

        warm_ps = ps_bc.tile([128, 512], F32, tag="bc", name="warm")
        n_warm = 144
        for i in range(n_warm):
            nc.tensor.matmul(
                warm_ps[:, 0:128], ones_bf[:], ones_bf[:],
                start=(i == 0), stop=(i == n_warm - 1),
            )
        warm_sink = const.tile([1, 1], F32, tag="warm_sink", name="warm_sink")
        nc.vector.tensor_copy(warm_sink[:], warm_ps[0:1, 0:1])

        sq = []
        for ci in range(2):
            t = const.tile([128, 2], F32, tag=f"sq{ci}", name=f"sq{ci}")  # [sum, sumsq]
            nc.vector.reduce_sum(t[:, 0:1], sqp[ci][:], axis=AX.X)
            nc.vector.reduce_sum(t[:, 1:2], sqq[ci][:], axis=AX.X)
            sq.append(t)

        gs_ps = ps_p.tile([8, 2], F32, tag="p", name="p")  # group [sum, sumsq]
        for ci in range(2):
            nc.tensor.matmul(
                gs_ps[:], mf_sb[:, ds(8 * ci, 8)], sq[ci][:],
                start=(ci == 0), stop=(ci == 1),
            )
        dum = const.tile([8, 1], F32, tag="dum", name="dum")
        nc.scalar.activation(dum[:], gs_ps[:, 0:1], AF.Sqrt, scale=0.0, bias=1.0)
        stats = const.tile([8, 2], F32, tag="stats", name="stats")  # [mean, rstd]
        tmp8 = const.tile([8, 2], F32, tag="tmp8", name="tmp8")  # [mean^2, E[x^2]]
        inv_ng = 1.0 / NG_ELEMS
        nc.vector.tensor_scalar_mul(stats[:, 0:1], gs_ps[:, 0:1], inv_ng)
        nc.vector.tensor_scalar_mul(tmp8[:, 1:2], gs_ps[:, 1:2], inv_ng)
        nc.vector.tensor_mul(tmp8[:, 0:1], stats[:, 0:1], stats[:, 0:1])
        var8 = const.tile([8, 1], F32, tag="var8", name="var8")
        nc.vector.tensor_sub(var8[:], tmp8[:, 1:2], tmp8[:, 0:1])
        nc.vector.tensor_scalar_add(var8[:], var8[:], EPS)
        sd8 = const.tile([8, 1], F32, tag="sd8", name="sd8")
        nc.scalar.activation(sd8[:], var8[:], AF.Sqrt)
        nc.vector.reciprocal(stats[:, 1:2], sd8[:])

        # broadcast rstd to channels; per-channel scale a = gamma * rstd
        m8 = const.tile([8, 1], F32, tag="m8", name="m8")
        nc.vector.tensor_mul(m8[:], stats[:, 0:1], stats[:, 1:2])
        a_sb = []
        for ci in range(2):
            ch_ps = ps_p.tile([128, 1], F32, tag="p", name="p")
            nc.tensor.matmul(
                ch_ps[:], mt_sb[:, ts(ci, 128)], stats[:, 1:2], start=True, stop=True
            )
            a_t = const.tile([128, 1], F32, tag=f"a{ci}", name=f"a{ci}")
            nc.vector.tensor_mul(a_t[:], gam_sb[:, ci : ci + 1], ch_ps[:])
            a_sb.append(a_t)

        # scale qkv weights by a (per input channel = partition)
        wts = []
        for ci in range(2):
            t = const.tile([128, 3 * C], BF, tag=f"wts{ci}", name=f"wts{ci}")
            if ci == 0:
                nc.scalar.activation(t[:], wt_raw[ci][:], AF.Copy, scale=a_sb[ci][:])
            else:
                nc.vector.tensor_scalar_mul(t[:], wt_raw[ci][:], a_sb[ci][:])
            wts.append(t)

        # ---- qkv projections (bias matmuls interleaved after the first 4
        # tiles so the PE isn't serialized on the tiny bias chain) ----
        q_sb, k_sb = [], []
        for ci in range(2):
            q_sb.append(big.tile([128, NQ], BF, tag=f"q{ci}", name=f"q{ci}"))
            k_sb.append(big.tile([128, N], BF, tag=f"k{ci}", name=f"k{ci}"))
        plans = [
            (0, q_sb[0], NQ), (1, q_sb[1], NQ),
            (2, k_sb[0], N), (3, k_sb[1], N),
        ]
        jobs = [(j, dst, nt) for j, dst, ncols in plans for nt in range(ncols // 512)]

        def qkv_mm(idx):
            j, dst, nt = jobs[idx]
            pool = ps_s if idx % 2 == 0 else ps_o
            ptag = "s" if idx % 2 == 0 else "o"
            ps = pool.tile([128, 512], F32, tag=ptag, name=ptag)
            for ci in range(2):
                nc.tensor.matmul(
                    ps[:], wts[ci][:, ts(j, 128)],
                    xs[ci][:, ts(nt, 512)],
                    start=(ci == 0), stop=(ci == 1),
                )
            return ps

        def qkv_evac(idx, ps):
            j, dst, nt = jobs[idx]
            if idx % 2 == 0:
                nc.scalar.activation(
                    dst[:, ts(nt, 512)], ps[:], AF.Identity,
                    bias=qb_eff[:, j : j + 1],
                )
            else:
                nc.vector.tensor_scalar_add(
                    dst[:, ts(nt, 512)], ps[:], qb_eff[:, j : j + 1]
                )

        qb_eff = const.tile([128, 6], F32, tag="qb_eff", name="qb_eff")
        head = [qkv_mm(i) for i in range(4)]

        # effective biases: cst - sum_g (mean_g*rstd_g) * G[g, :]
        bb_ps = ps_p.tile([128, 6], F32, tag="p", name="p")
        for j in range(6):
            nc.tensor.matmul(
                bb_ps[:, j : j + 1], g_sb[:, ts(j, 128)], m8[:],
                start=True, stop=True,
            )
        nc.vector.tensor_sub(qb_eff[:], cstq_sb[:], bb_ps[:])
        pbps = ps_p.tile([128, 2], F32, tag="p", name="p")
        for ob in range(2):
            nc.tensor.matmul(
                pbps[:, ob : ob + 1], pg_sb[:, ts(ob, 128)], m8[:],
                start=True, stop=True,
            )
        pb_eff = const.tile([128, 2], F32, tag="pb_eff", name="pb_eff")
        nc.vector.tensor_sub(pb_eff[:], cstp_sb[:], pbps[:])

        for i in range(4):
            qkv_evac(i, head[i])
        for idx in range(4, len(jobs)):
            ps = qkv_mm(idx)
            qkv_evac(idx, ps)

        # v^T: (nk, v-channel) layout, no bias (folded into proj bias)
        vt_sb = big.tile([128, 32, 256], BF, tag="vt", name="vt")
        for t in range(32):
            pool = ps_s if t % 2 == 0 else ps_o
            ptag = "s" if t % 2 == 0 else "o"
            ps = pool.tile([128, 512], F32, tag=ptag, name=ptag)
            for ci in range(2):
                nc.tensor.matmul(
                    ps[:, 0:256], xs[ci][:, ts(t, 128)],
                    wts[ci][:, ds(512, 256)],
                    start=(ci == 0), stop=(ci == 1),
                )
            if t % 2 == 0:
                nc.vector.tensor_copy(vt_sb[:, t, :], ps[:, 0:256])
            else:
                nc.scalar.activation(vt_sb[:, t, :], ps[:, 0:256], AF.Copy)

        # x + pb_eff precomputed for the residual tail
        xpb = []
        for ob in range(2):
            xr_t = big.tile([128, NQ], F32, tag=f"xr{ob}", name=f"xr{ob}")
            nc.sync.dma_start(xr_t[:], xr_d[ts(ob, 128), :])
            t = big.tile([128, NQ], F32, tag=f"xpb{ob}", name=f"xpb{ob}")
            nc.scalar.activation(
                t[:], xr_t[:], AF.Identity, bias=pb_eff[:, ob : ob + 1]
            )
            xpb.append(t)

        # ---- attention + proj, per block of 512 queries ----
        for nqb in range(4):
            o_ps = [ps_o.tile([128, 512], F32, tag="o", name="o") for _ in range(2)]
            acc = [accp.tile([128, 512], BF, tag="acc", name="acc") for _ in range(2)]
            for t in range(32):
                s_ps = ps_s.tile([128, 512], F32, tag="s", name="s")
                for ci in range(2):
                    nc.tensor.matmul(
                        s_ps[:], k_sb[ci][:, ts(t, 128)],
                        q_sb[ci][:, ts(nqb, 512)],
                        start=(ci == 0), stop=(ci == 1),
                    )
                e_t = expp.tile([128, 512], BF, tag="e", name="e")
                nc.scalar.activation(e_t[:], s_ps[:], AF.Exp, scale=SCALE)
                first, last = (t == 0), (t == 31)
                for c2 in range(2):
                    nc.tensor.matmul(
                        o_ps[c2][:], vt_sb[:, t, ds(128 * c2, 128)],
                        e_t[:], start=first, stop=last,
                    )
                # denominator partials on DVE (two chains to halve latency)
                ef = e_t[:]
                eng = nc.vector if t % 2 == 0 else nc.gpsimd
                if t < 2:
                    eng.tensor_copy(acc[t][:], ef)
                else:
                    a = acc[t % 2]
                    eng.tensor_add(a[:], a[:], ef)
            nc.vector.tensor_add(acc[0][:], acc[0][:], acc[1][:])
            # unnormalized attention out -> SBUF, proj (normalization commutes)
            o_t = [osb.tile([128, 512], FR, tag="ob", name="ob") for _ in range(2)]
            nc.scalar.activation(o_t[0][:], o_ps[0][:], AF.Copy)
            nc.vector.tensor_copy(o_t[1][:], o_ps[1][:])
            p_ps = []
            for ob in range(2):
                pp = ps_p.tile([128, 512], F32, tag="p", name="p")
                for c2 in range(2):
                    nc.tensor.matmul(
                        pp[:], wpt_fr[c2][:, ts(ob, 128)], o_t[c2][:],
                        start=(c2 == 0), stop=(c2 == 1),
                    )
                p_ps.append(pp)
            # denominators: all-ones matmul = column sums broadcast to all
            # partitions in one shot; reciprocal on the full 128-lane tile
            bc_ps = ps_bc.tile([128, 512], F32, tag="bc", name="bc")
            nc.tensor.matmul(bc_ps[:], ones_bf[:], acc[0][:], start=True, stop=True)
            bc_sb = osb.tile([128, 512], F32, tag="bcs", name="bcs")
            nc.vector.reciprocal_approx_fast(bc_sb[:], bc_ps[:])
            for ob in range(2):
                r_t = outp.tile([128, 512], F32, tag="r", name="r")
                nc.vector.tensor_mul(r_t[:], p_ps[ob][:], bc_sb[:])
                f_t = outp.tile([128, 512], F32, tag="f", name="f")
                nc.vector.tensor_add(f_t[:], r_t[:], xpb[ob][:, ts(nqb, 512)])
                nc.sync.dma_start(out_d[ts(ob, 128), ts(nqb, 512)], f_t[:])


def _build():
    global _CACHED_NC
    if _CACHED_NC is not None:
        return _CACHED_NC
    nc = bacc.Bacc("TRN2", debug=False, target_bir_lowering=False)
    x_d = nc.dram_tensor("x", [C, N], BF, kind="ExternalInput").ap()
    xr_d = nc.dram_tensor("xr", [C, NQ], F32, kind="ExternalInput").ap()
    wt_d = nc.dram_tensor("wt", [C, 3 * C], BF, kind="ExternalInput").ap()
    wpt_d = nc.dram_tensor("wpt", [C, C], F32, kind="ExternalInput").ap()
    g_d = nc.dram_tensor("gmat", [8, 3 * C], F32, kind="ExternalInput").ap()
    pg_d = nc.dram_tensor("pgmat", [8, C], F32, kind="ExternalInput").ap()
    cstq_d = nc.dram_tensor("cstq", [128, 6], F32, kind="ExternalInput").ap()
    cstp_d = nc.dram_tensor("cstp", [128, 2], F32, kind="ExternalInput").ap()
    gam_d = nc.dram_tensor("gam", [128, 2], F32, kind="ExternalInput").ap()
    mf_d = nc.dram_tensor("maskf", [128, 16], F32, kind="ExternalInput").ap()
    mt_d = nc.dram_tensor("maskt", [8, 256], F32, kind="ExternalInput").ap()
    out_d = nc.dram_tensor("out", [C, NQ], F32, kind="ExternalOutput").ap()
    aps = (x_d, xr_d, wt_d, wpt_d, g_d, pg_d, cstq_d, cstp_d, gam_d, mf_d, mt_d, out_d)
    with tile.TileContext(nc) as tc:
        _emit(tc, aps)
    nc.compile()
    _CACHED_NC = nc
    return nc


def kernel(x, gn_gamma, gn_beta, qkv_w, qkv_b, proj_w, proj_b):
    global LAST_RESULT
    x = np.asarray(x, dtype=np.float32)
    gn_gamma = np.asarray(gn_gamma, dtype=np.float32)
    gn_beta = np.asarray(gn_beta, dtype=np.float32)
    qkv_w = np.asarray(qkv_w, dtype=np.float32)
    qkv_b = np.asarray(qkv_b, dtype=np.float32)
    proj_w = np.asarray(proj_w, dtype=np.float32)
    proj_b = np.asarray(proj_b, dtype=np.float32)

    xf = np.ascontiguousarray(x.reshape(B, C, N))
    wt = np.ascontiguousarray(qkv_w.T).astype(ml_dtypes.bfloat16)  # (C, 3C)
    wpt = np.ascontiguousarray(proj_w.T)  # (C, C) fp32 (fp32r on device)
    gam = np.ascontiguousarray(gn_gamma.reshape(2, 128).T)

    # host-folded bias constants:
    #   b_eff = cst - sum_g (mean_g * rstd_g) * G[g, :]
    # with G[g, o] = sum_{c in g} qkv_w[o, c] * gamma_c and
    # cst = qkv_b + qkv_w @ beta. Proj bias gets the same treatment through
    # proj_w (softmax rows sum to 1, so the v-bias passes through attention).
    grp_size = C // G
    gmat = np.zeros((G, 3 * C), np.float32)
    for g in range(G):
        sl = slice(g * grp_size, (g + 1) * grp_size)
        gmat[g] = qkv_w[:, sl] @ gn_gamma[sl]
    cst_qkv = qkv_b + qkv_w @ gn_beta  # (768,)
    pgmat = np.ascontiguousarray(gmat[:, 2 * C :] @ proj_w.T)  # (8, 256)
    cst_pb = proj_b + proj_w @ cst_qkv[2 * C :]  # (256,)
    cstq = np.ascontiguousarray(cst_qkv.reshape(6, 128).T)
    cstp = np.ascontiguousarray(cst_pb.reshape(2, 128).T)

    # group-membership masks (channels-per-partition <-> groups)
    ch = np.arange(C)
    grp = ch // (C // G)  # (256,)
    mf = np.zeros((128, 16), np.float32)  # [c_lo, ci*8 + g]
    for ci in range(2):
        for c_lo in range(128):
            mf[c_lo, ci * 8 + grp[ci * 128 + c_lo]] = 1.0
    mt = np.zeros((8, 256), np.float32)  # [g, c]
    mt[grp, ch] = 1.0

    in_maps = []
    for core in range(N_CORES):
        b, h = core // 2, core % 2
        xb = xf[b]
        if h:
            xc = np.ascontiguousarray(np.concatenate([xb[:, NQ:], xb[:, :NQ]], axis=1))
        else:
            xc = xb
        in_maps.append(
            {
                "x": xc.astype(ml_dtypes.bfloat16),
                "xr": np.ascontiguousarray(xc[:, :NQ]),
                "wt": wt, "wpt": wpt, "gmat": gmat, "pgmat": pgmat,
                "cstq": cstq, "cstp": cstp,
                "gam": gam, "maskf": mf, "maskt": mt,
            }
        )

    nc = _build()
    res = run_bass_kernel_spmd(nc, in_maps, core_ids=list(range(N_CORES)))
    LAST_RESULT = res

    out = np.empty((B, C, N), np.float32)
    for core in range(N_CORES):
        b, h = core // 2, core % 2
        out[b][:, h * NQ : (h + 1) * NQ] = res.results[core]["out"]
    return out.reshape(B, C, D, H, W)
